# revision 31
# baseline (speedup 1.0000x reference)
"""Multi-head self-attention (RoPE, causal) Trainium2 kernel, v2.1.

Tensor-parallel over heads: 16 heads / 8 cores = 2 heads per core
(Megatron-style: Wq/Wk/Wv sharded on output dim, Wo on input dim).
Each core computes a full [S, D] partial of the output projection;
the host sums the 8 partials.

v2 changes vs v1:
- fp16 datapath (x, weights, q/k/v, e, attn, output partials); PSUM f32.
- PV computed transposed: out[q,d] = sum_k e[k,q] v[k,d] with the exp
  tile as the (FWL-eligible, 128-col) stationary and [v|1] as the
  65-col moving operand -> half the PV matmul columns, and the softmax
  denominator lands as a per-query PSUM column.
- Normalization is a per-partition scalar multiply on DVE (no broadcast
  matmuls).
- Chunk finish work (normalize/transpose/out-proj) is interleaved into
  the next chunk's attention groups to keep ACT busy continuously.
- Scores run two k-tiles ahead of PV so ACT always has the next exp
  queued and never starves while the PE waits on PV dependencies.

Self-contained: hardcodes all shapes; no sibling imports.
"""

import numpy as np

S = 4096
D = 1024
DK = 64
NCORES = 8
THETA = 10000.0
CH = 512          # sequence chunk (matmul moving free dim)
NCH = S // CH     # 8 chunks
VS = 132          # v_sb column stride per s-tile: [vA(64) 1A vB(64) 1B pad(2)]

_CACHE = {}


# ---------------------------------------------------------------------------
# host-side layout helpers
# ---------------------------------------------------------------------------

def _rope_perm64():
    """Permutation of a head's 64 dims so RoPE pairs line up for a
    32-lane stream_shuffle: quadrant q (32 partitions) holds pairs
    16q..16q+15 as [evens(16) | odds(16)]."""
    perm = np.zeros(64, np.int64)
    for d in range(64):
        j, odd = d // 2, d % 2
        pos = 32 * (j // 16) + 16 * odd + (j % 16)
        perm[pos] = d
    return perm


def _trig_tables():
    # partition p: pair index = 16*((p//32)%2) + p%16 ; odd slot if p%32 >= 16
    p = np.arange(128)
    pair = 16 * ((p // 32) % 2) + (p % 16)
    odd = (p % 32) >= 16
    inv_freq = THETA ** (-2.0 * pair / DK)           # [128]
    pos = np.arange(S, dtype=np.float64)
    ang = pos[None, :] * inv_freq[:, None]           # [128, S]
    cos = np.cos(ang).astype(np.float32)
    sin = (np.where(odd[:, None], 1.0, -1.0) * np.sin(ang)).astype(np.float32)
    return cos, sin


def _host_prep(x, Wq, Wk, Wv, Wo):
    x = np.asarray(x, dtype=np.float32).reshape(S, D)
    Wq = np.asarray(Wq, dtype=np.float32)
    Wk = np.asarray(Wk, dtype=np.float32)
    Wv = np.asarray(Wv, dtype=np.float32)
    Wo = np.asarray(Wo, dtype=np.float32)

    xT = np.ascontiguousarray(x.T).astype(np.float16)          # [D, S]
    cos, sin = _trig_tables()
    tri = (np.arange(128)[None, :] >= np.arange(128)[:, None])
    tri = tri.astype(np.float16)

    perm = _rope_perm64()
    in_maps = []
    for c in range(NCORES):
        hA, hB = 2 * c, 2 * c + 1
        rows_qk = np.concatenate([64 * hA + perm, 64 * hB + perm])
        rows_v = np.arange(128 * c, 128 * c + 128)
        wq_c = np.ascontiguousarray(Wq[rows_qk, :].T).astype(np.float16)
        wk_c = np.ascontiguousarray(Wk[rows_qk, :].T).astype(np.float16)
        wv_c = np.ascontiguousarray(Wv[rows_v, :].T).astype(np.float16)
        wo_c = np.ascontiguousarray(Wo[:, rows_v].T).astype(np.float16)
        in_maps.append({
            "xT": xT, "wq": wq_c, "wk": wk_c, "wv": wv_c, "wo": wo_c,
            "cos": cos, "sin": sin, "tri": tri,
            "ones": np.ones((128, 64), np.float16),
            "iden": np.eye(128, dtype=np.float32),
            "iden16": np.eye(128, dtype=np.float16),
        })
    return in_maps


# ---------------------------------------------------------------------------
# device program
# ---------------------------------------------------------------------------

def _emit(tc, out, xT, wq, wk, wv, wo, cos, sin, tri, ones, iden, iden16,
          repeats=1):
    import concourse.mybir as mybir

    nc = tc.nc
    f32 = mybir.dt.float32
    f32r = mybir.dt.float32r
    f16 = mybir.dt.float16
    AF = mybir.ActivationFunctionType
    OP = mybir.AluOpType
    SWAP_MASK = [(i + 16) % 32 for i in range(32)]

    with (
        tc.tile_pool(name="consts", bufs=1) as consts,
        tc.tile_pool(name="persist", bufs=1) as persist,
        tc.tile_pool(name="xtp", bufs=2) as xtp,
        tc.tile_pool(name="rope", bufs=3) as ropep,
        tc.tile_pool(name="trig", bufs=2) as trigp,
        tc.tile_pool(name="expp", bufs=5) as expp,
        tc.tile_pool(name="small", bufs=4) as smallp,
        tc.tile_pool(name="attn", bufs=10) as attnp,
        tc.tile_pool(name="outTp", bufs=4) as outTp,
        tc.tile_pool(name="stagep", bufs=3) as stagep,
        tc.tile_pool(name="ps_s", bufs=2, space="PSUM") as ps_scores,
        tc.tile_pool(name="ps_pv", bufs=1, space="PSUM") as ps_pv,
        tc.tile_pool(name="ps_m", bufs=2, space="PSUM") as ps_misc,
    ):
        pstate = {}

        def fetch_chunk(j):
            """Issue chunk j's x and trig DMAs."""
            jsl = slice(j * CH, (j + 1) * CH)
            xt = xtp.tile([128, 8 * CH], f16, tag="xt")
            pstate[("xt", j)] = xt
            nc.sync.dma_start(
                out=xt.rearrange("p (t s) -> p t s", s=CH),
                in_=xT[:, jsl].rearrange("(t p) s -> p t s", p=128),
            )
            cs = trigp.tile([128, CH], f32, tag="cs")
            nc.sync.dma_start(out=cs, in_=cos[:, jsl])
            sn = trigp.tile([128, CH], f32, tag="sn")
            nc.sync.dma_start(out=sn, in_=sin[:, jsl])
            pstate[("trig", j)] = (cs, sn)

        # ---- constants (critical-path DMAs first) ----------------------
        wq_sb = consts.tile([128, 1024], f16)
        wk_sb = consts.tile([128, 1024], f16)
        wv_sb = consts.tile([128, 1024], f16)
        for sb, dram in ((wq_sb, wq), (wk_sb, wk), (wv_sb, wv)):
            nc.sync.dma_start(
                out=sb.rearrange("p (t m) -> p t m", m=128),
                in_=dram.rearrange("(t p) m -> p t m", p=128),
            )
        wo_sb = consts.tile([128, 1024], f16)
        nc.sync.dma_start(out=wo_sb, in_=wo)
        tri_sb = consts.tile([128, 128], f16)
        nc.sync.dma_start(out=tri_sb, in_=tri)
        zero_sb = consts.tile([128, 128], f16)
        nc.vector.memset(zero_sb, 0.0)
        # preload the exp activation table while the weight DMAs run, so
        # the ~2.7us ACT_TABLE_LOAD is off the first chunk's critical path
        pre = smallp.tile([1, 64], f32, tag="pre")
        nc.scalar.activation(pre, zero_sb[0:1, 0:64],
                             AF.Exp, scale=1.0)
        id_sb = consts.tile([128, 128], f32r)
        nc.sync.dma_start(out=id_sb, in_=iden)
        id16_sb = consts.tile([128, 128], f16)
        nc.sync.dma_start(out=id16_sb, in_=iden16)

        qT_sb = persist.tile([128, S], f16)  # RoPE'd q, [dk(2 heads), s]
        kT_sb = persist.tile([128, S], f16)
        v_sb = persist.tile([128, 32 * VS], f16)
        vv = v_sb.rearrange("p (t c) -> p t c", c=VS)
        ones32 = ones.rearrange("p (t o) -> p t o", o=2)[:, 0:32, :]
        nc.sync.dma_start(out=vv[:, :, 64:65], in_=ones32[:, :, 0:1])
        nc.sync.dma_start(out=vv[:, :, 129:130], in_=ones32[:, :, 1:2])

        def rope(ps, dst, j):
            """dst = ps * cos + swap(ps) * sin  (chunk j)."""
            cs, sn = pstate[("trig", j)]
            sw = ropep.tile([128, CH], f32, tag="sw")
            nc.vector.stream_shuffle(sw, ps, SWAP_MASK)
            t1 = ropep.tile([128, CH], f32, tag="t1")
            nc.vector.tensor_tensor(t1, ps, cs, OP.mult)
            t2 = ropep.tile([128, CH], f32, tag="t2")
            nc.vector.tensor_tensor(t2, sw, sn, OP.mult)
            nc.vector.tensor_tensor(dst, t1, t2, OP.add)

        def proj_u1(j):
            """q projection + RoPE(q)."""
            jsl = slice(j * CH, (j + 1) * CH)
            if ("xt", j) not in pstate:
                fetch_chunk(j)
            xt = pstate[("xt", j)]
            psq = ps_misc.tile([128, CH], f32, tag="m")
            for t in range(8):
                nc.tensor.matmul(psq, wq_sb[:, t * 128:(t + 1) * 128],
                                 xt[:, t * CH:(t + 1) * CH],
                                 start=(t == 0), stop=(t == 7))
            rope(psq, qT_sb[:, jsl], j)

        def proj_u2(j):
            """k projection + RoPE(k)."""
            jsl = slice(j * CH, (j + 1) * CH)
            xt = pstate[("xt", j)]
            psk = ps_misc.tile([128, CH], f32, tag="m")
            for t in range(8):
                nc.tensor.matmul(psk, wk_sb[:, t * 128:(t + 1) * 128],
                                 xt[:, t * CH:(t + 1) * CH],
                                 start=(t == 0), stop=(t == 7))
            rope(psk, kT_sb[:, jsl], j)
            del pstate[("trig", j)]

        def proj_u3(j):
            """vT projection + copy to SBUF."""
            xt = pstate.pop(("xt", j))
            psvT = ps_misc.tile([128, CH], f32, tag="m")
            for t in range(8):
                nc.tensor.matmul(psvT, wv_sb[:, t * 128:(t + 1) * 128],
                                 xt[:, t * CH:(t + 1) * CH],
                                 start=(t == 0), stop=(t == 7))
            vt = ropep.tile([128, CH], f16, tag="vt")
            nc.vector.tensor_copy(vt, psvT)
            pstate[("vt", j)] = vt

        def proj_u4(j):
            """PE-transpose vT -> v tiles in v_sb (with ones columns)."""
            vt = pstate.pop(("vt", j))
            pst = ps_misc.tile([128, CH], f16, tag="m")
            for st in range(4):
                nc.tensor.transpose(pst[:, st * 128:(st + 1) * 128],
                                    vt[:, st * 128:(st + 1) * 128], id16_sb)
            for st in range(4):
                cb = (4 * j + st) * VS
                nc.vector.tensor_copy(v_sb[:, cb:cb + 64],
                                      pst[:, st * 128:st * 128 + 64])
                nc.vector.tensor_copy(v_sb[:, cb + 65:cb + 129],
                                      pst[:, st * 128 + 64:st * 128 + 128])

        def fin_norm(j):
            """Normalize chunk j's PV output: attnN = pv[:, :64] / den.
            Runs only after ALL of chunk j's PV accumulation stopped —
            reading a PSUM bank while the PE still accumulates other
            columns of the same bank corrupts data on hardware."""
            pvTA, pvTB = pstate.pop(("pvT", j))
            rcs = []
            for pvT in (pvTA, pvTB):
                dens = pvT.rearrange("p (s c) -> p s c", c=65)[:, :, 64:65]
                rc = smallp.tile([128, 4, 1], f32, tag="rc")
                nc.vector.reciprocal_approx_fast(out=rc, in_=dens)
                rcs.append(rc)
            for sub in range(4):
                an = attnp.tile([128, 128], f16, tag="an")
                for h, pvT in ((0, pvTA), (1, pvTB)):
                    nc.vector.tensor_scalar(
                        an[:, h * 64:h * 64 + 64],
                        pvT[:, sub * 65:sub * 65 + 64],
                        rcs[h][:, sub, :], None, OP.mult)
                pstate[("an", j, sub)] = an

        def fin_tr(j, st):
            """Transpose attnN for 128 output rows back to [d, q] on the
            DMA xbar transpose engine -- frees a PE transpose and a DVE
            PSUM->SBUF copy per subtile, and a ps_misc bank rotation."""
            an = pstate.pop(("an", j, st))
            outTs = outTp.tile([128, 128], f16, tag="oT")
            nc.sync.dma_start_transpose(out=outTs, in_=an)
            pstate[("oT", j, st)] = outTs

        def fin_out(j, st):
            """Out-projection + store for 128 output rows of chunk j."""
            outTs = pstate.pop(("oT", j, st))
            stg = stagep.tile([128, 1024], f16, tag="stg")
            for oc in range(2):
                ops = ps_misc.tile([128, CH], f32, tag="m")
                nc.tensor.matmul(ops, outTs, wo_sb[:, oc * 512:(oc + 1) * 512],
                                 start=True, stop=True)
                nc.vector.tensor_copy(stg[:, oc * 512:(oc + 1) * 512], ops)
            r0 = j * CH + st * 128
            nc.sync.dma_start(out=out[r0:r0 + 128, :], in_=stg)

        def fin_st(j, st):
            fin_tr(j, st)
            fin_out(j, st)

        def attn_scores(j, t):
            """Scores for k-tile t against chunk j's queries (heads A||B via
            row tiling, packed side by side in one PSUM tile), then exp on
            ACT into one fp16 SBUF tile (single ACTIVATE when non-diag).
            Idempotent per (chunk-local) tile so transition pre-emission and
            the in-loop emission compose."""
            if ("e", t) in pstate:
                return
            jsl = slice(j * CH, (j + 1) * CH)
            off = max(0, 128 * (t - 4 * j))
            diag = t >= 4 * j
            ksl = slice(t * 128, (t + 1) * 128)
            sAB = ps_scores.tile([128, 2 * CH], f32, tag="s")
            nc.tensor.matmul(sAB[:, off:CH], kT_sb[0:64, ksl],
                             qT_sb[0:64, jsl][:, off:CH],
                             start=True, stop=True)
            nc.tensor.matmul(sAB[:, CH + off:2 * CH], kT_sb[64:128, ksl],
                             qT_sb[64:128, jsl][:, off:CH],
                             start=True, stop=True)
            eAB = expp.tile([128, 2 * CH], f16, tag="e")
            if not diag:
                nc.scalar.activation(eAB, sAB, AF.Exp, scale=0.125)
            else:
                nc.scalar.activation(eAB[:, off:CH], sAB[:, off:CH], AF.Exp,
                                     scale=0.125)
                nc.scalar.activation(eAB[:, CH + off:2 * CH],
                                     sAB[:, CH + off:2 * CH], AF.Exp,
                                     scale=0.125)
                for hb in (0, CH):
                    tm = slice(hb + off, hb + off + 128)
                    nc.vector.tensor_tensor(eAB[:, tm], eAB[:, tm], tri_sb,
                                            OP.mult)
            pstate[("e", t)] = eAB

        def attn_pvt(j, t, pvTA, pvTB):
            """PV-transposed accumulation for k-tile t into chunk j's
            per-query-subtile PSUM accumulators."""
            eAB = pstate.pop(("e", t))
            sub0 = max(0, t - 4 * j)
            for hb, pvT, vcol in ((0, pvTA, 0), (CH, pvTB, 65)):
                for sub in range(sub0, 4):
                    nc.tensor.matmul(
                        pvT[:, sub * 65:sub * 65 + 65],
                        eAB[:, hb + sub * 128:hb + sub * 128 + 128],
                        v_sb[:, t * VS + vcol:t * VS + vcol + 65],
                        start=False, stop=(t == 4 * j + sub),
                        skip_group_check=True)

        PROJ_UNITS = (proj_u1, proj_u2, proj_u3, proj_u4)

        for rep in range(repeats):
            for u in PROJ_UNITS:
                u(0)
            for j in range(NCH):
                pvTA = ps_pv.tile([128, 260], f32, tag="pvA")
                pvTB = ps_pv.tile([128, 260], f32, tag="pvB")
                pstate[("pvT", j)] = (pvTA, pvTB)
                # zero each PV psum accumulator once with an in-stream PE
                # matmul (zero stationary); all PV matmuls then accumulate
                # with start=False (a per-subtile start would zero the whole
                # 2KB bank region and clobber sibling subtile accumulators).
                for pvT in (pvTA, pvTB):
                    nc.tensor.matmul(pvT, zero_sb, wq_sb[:, 0:260],
                                     start=True, stop=False,
                                     skip_group_check=True)
                tiles = list(range(4 * j + 4))
                units = []
                if j + 1 < NCH:
                    # q/k projections first so the next chunk's scores can
                    # issue immediately at the transition
                    units.append(lambda jj=j + 1: proj_u1(jj))
                    units.append(lambda jj=j + 1: proj_u2(jj))
                if j > 0:
                    for st in range(4):
                        units.append(lambda jj=j - 1, ss=st: fin_st(jj, ss))
                if j + 1 < NCH:
                    units.append(lambda jj=j + 1: proj_u3(jj))
                    units.append(lambda jj=j + 1: proj_u4(jj))
                ng = len(tiles)
                done = 0
                attn_scores(j, 0)
                if ng > 1:
                    attn_scores(j, 1)
                for ti, t in enumerate(tiles):
                    # scores run two tiles ahead of this tile's PV so ACT
                    # always has the next exp queued and never starves
                    # while the PE waits on PV dependencies.
                    if t + 2 < ng:
                        attn_scores(j, t + 2)
                    attn_pvt(j, t, pvTA, pvTB)
                    want = (ti + 1) * len(units) // ng
                    while done < want:
                        units[done]()
                        done += 1
                # normalize as soon as this chunk's accumulation is complete:
                # frees the PV psum banks so the next chunk's zeroing matmuls
                # don't stall the PE queue on a late cross-engine WAR.
                fin_norm(j)
                # pre-emit the next chunk's first two score tiles (q/k were
                # projected mid-chunk) so ACT rolls into the next chunk's
                # exp stream with no transition gap.
                if j + 1 < NCH:
                    attn_scores(j + 1, 0)
                    attn_scores(j + 1, 1)

            for st in range(4):
                fin_tr(NCH - 1, st)
            for st in range(4):
                fin_out(NCH - 1, st)


def _build(repeats=1):
    import concourse.mybir as mybir
    import concourse.tile as tile
    from concourse import bacc

    f32 = mybir.dt.float32
    f16 = mybir.dt.float16
    nc = bacc.Bacc("TRN2", target_bir_lowering=False, debug=False,
                   num_devices=NCORES)
    aps = {}
    for name, shape in (
        ("xT", [D, S]), ("wq", [D, 128]), ("wk", [D, 128]), ("wv", [D, 128]),
        ("wo", [128, D]), ("ones", [128, 64]), ("tri", [128, 128]),
    ):
        aps[name] = nc.dram_tensor(name, shape, f16, kind="ExternalInput").ap()
    for name, shape in (
        ("cos", [128, S]), ("sin", [128, S]),
    ):
        aps[name] = nc.dram_tensor(name, shape, f32, kind="ExternalInput").ap()
    aps["iden"] = nc.dram_tensor(
        "iden", [128, 128], mybir.dt.float32r, kind="ExternalInput").ap()
    aps["iden16"] = nc.dram_tensor(
        "iden16", [128, 128], f16, kind="ExternalInput").ap()
    out_ap = nc.dram_tensor("out", [S, D], f16, kind="ExternalOutput").ap()

    with tile.TileContext(nc) as tc:
        _emit(tc, out_ap, aps["xT"], aps["wq"], aps["wk"], aps["wv"],
              aps["wo"], aps["cos"], aps["sin"], aps["tri"], aps["ones"],
              aps["iden"], aps["iden16"], repeats=repeats)
    nc.compile()
    return nc


def kernel(x, Wq, Wk, Wv, Wo):
    from concourse.bass_utils import run_bass_kernel_spmd

    if "nc" not in _CACHE:
        _CACHE["nc"] = _build()
    nc = _CACHE["nc"]

    in_maps = _host_prep(x, Wq, Wk, Wv, Wo)
    res = run_bass_kernel_spmd(nc, in_maps, core_ids=list(range(NCORES)))
    acc = np.zeros((S, D), dtype=np.float64)
    for r in res.results:
        acc += r["out"].astype(np.float64)
    return acc.astype(np.float32).reshape(1, S, D)



# revision 33
# speedup vs baseline: 1.1515x; 1.1515x over previous
"""Multi-head self-attention (RoPE, causal) Trainium2 kernel, v2.1.

Tensor-parallel over heads: 16 heads / 8 cores = 2 heads per core
(Megatron-style: Wq/Wk/Wv sharded on output dim, Wo on input dim).
Each core computes a full [S, D] partial of the output projection;
the host sums the 8 partials.

v2 changes vs v1:
- fp16 datapath (x, weights, q/k/v, e, attn, output partials); PSUM f32.
- PV computed transposed: out[q,d] = sum_k e[k,q] v[k,d] with the exp
  tile as the (FWL-eligible, 128-col) stationary and [v|1] as the
  65-col moving operand -> half the PV matmul columns, and the softmax
  denominator lands as a per-query PSUM column.
- Normalization is a per-partition scalar multiply on DVE (no broadcast
  matmuls).
- Chunk finish work (normalize/transpose/out-proj) is interleaved into
  the next chunk's attention groups to keep ACT busy continuously.
- Scores run two k-tiles ahead of PV so ACT always has the next exp
  queued and never starves while the PE waits on PV dependencies.

Self-contained: hardcodes all shapes; no sibling imports.
"""

import numpy as np

S = 4096
D = 1024
DK = 64
NCORES = 8
THETA = 10000.0
CH = 512          # sequence chunk (matmul moving free dim)
NCH = S // CH     # 8 chunks
VS = 132          # v_sb column stride per s-tile: [vA(64) 1A vB(64) 1B pad(2)]

_CACHE = {}


# ---------------------------------------------------------------------------
# host-side layout helpers
# ---------------------------------------------------------------------------

def _rope_perm64():
    """Permutation of a head's 64 dims so RoPE pairs line up for a
    32-lane stream_shuffle: quadrant q (32 partitions) holds pairs
    16q..16q+15 as [evens(16) | odds(16)]."""
    perm = np.zeros(64, np.int64)
    for d in range(64):
        j, odd = d // 2, d % 2
        pos = 32 * (j // 16) + 16 * odd + (j % 16)
        perm[pos] = d
    return perm


def _trig_tables():
    # partition p: pair index = 16*((p//32)%2) + p%16 ; odd slot if p%32 >= 16
    p = np.arange(128)
    pair = 16 * ((p // 32) % 2) + (p % 16)
    odd = (p % 32) >= 16
    inv_freq = THETA ** (-2.0 * pair / DK)           # [128]
    pos = np.arange(S, dtype=np.float64)
    ang = pos[None, :] * inv_freq[:, None]           # [128, S]
    cos = np.cos(ang).astype(np.float32)
    sin = (np.where(odd[:, None], 1.0, -1.0) * np.sin(ang)).astype(np.float32)
    return cos, sin


def _host_prep(x, Wq, Wk, Wv, Wo):
    x = np.asarray(x, dtype=np.float32).reshape(S, D)
    Wq = np.asarray(Wq, dtype=np.float32)
    Wk = np.asarray(Wk, dtype=np.float32)
    Wv = np.asarray(Wv, dtype=np.float32)
    Wo = np.asarray(Wo, dtype=np.float32)

    xT = np.ascontiguousarray(x.T).astype(np.float16)          # [D, S]
    cos, sin = _trig_tables()
    tri = (np.arange(128)[None, :] >= np.arange(128)[:, None])
    tri = tri.astype(np.float16)

    perm = _rope_perm64()
    in_maps = []
    for c in range(NCORES):
        hA, hB = 2 * c, 2 * c + 1
        rows_qk = np.concatenate([64 * hA + perm, 64 * hB + perm])
        rows_v = np.arange(128 * c, 128 * c + 128)
        wq_c = np.ascontiguousarray(Wq[rows_qk, :].T).astype(np.float16)
        wk_c = np.ascontiguousarray(Wk[rows_qk, :].T).astype(np.float16)
        wv_c = np.ascontiguousarray(Wv[rows_v, :].T).astype(np.float16)
        wo_c = np.ascontiguousarray(Wo[:, rows_v].T).astype(np.float16)
        in_maps.append({
            "xT": xT, "wq": wq_c, "wk": wk_c, "wv": wv_c, "wo": wo_c,
            "cos": cos, "sin": sin, "tri": tri,
            "ones": np.ones((128, 64), np.float16),
            "iden": np.eye(128, dtype=np.float32),
            "iden16": np.eye(128, dtype=np.float16),
        })
    return in_maps


# ---------------------------------------------------------------------------
# device program
# ---------------------------------------------------------------------------

def _emit(tc, out, xT, wq, wk, wv, wo, cos, sin, tri, ones, iden, iden16,
          repeats=1):
    import concourse.mybir as mybir

    nc = tc.nc
    f32 = mybir.dt.float32
    f32r = mybir.dt.float32r
    f16 = mybir.dt.float16
    AF = mybir.ActivationFunctionType
    OP = mybir.AluOpType
    SWAP_MASK = [(i + 16) % 32 for i in range(32)]

    with (
        tc.tile_pool(name="consts", bufs=1) as consts,
        tc.tile_pool(name="persist", bufs=1) as persist,
        tc.tile_pool(name="xtp", bufs=2) as xtp,
        tc.tile_pool(name="rope", bufs=3) as ropep,
        tc.tile_pool(name="trig", bufs=2) as trigp,
        tc.tile_pool(name="expp", bufs=5) as expp,
        tc.tile_pool(name="small", bufs=4) as smallp,
        tc.tile_pool(name="attn", bufs=10) as attnp,
        tc.tile_pool(name="outTp", bufs=4) as outTp,
        tc.tile_pool(name="stagep", bufs=3) as stagep,
        tc.tile_pool(name="ps_s", bufs=2, space="PSUM") as ps_scores,
        tc.tile_pool(name="ps_pv", bufs=1, space="PSUM") as ps_pv,
        tc.tile_pool(name="ps_m", bufs=2, space="PSUM") as ps_misc,
    ):
        pstate = {}

        def fetch_chunk(j):
            """Issue chunk j's x and trig DMAs."""
            if ("xt", j) in pstate:
                return
            jsl = slice(j * CH, (j + 1) * CH)
            xt = xtp.tile([128, 8 * CH], f16, tag="xt")
            pstate[("xt", j)] = xt
            nc.sync.dma_start(
                out=xt.rearrange("p (t s) -> p t s", s=CH),
                in_=xT[:, jsl].rearrange("(t p) s -> p t s", p=128),
            )
            cs = trigp.tile([128, CH], f32, tag="cs")
            nc.sync.dma_start(out=cs, in_=cos[:, jsl])
            sn = trigp.tile([128, CH], f32, tag="sn")
            nc.sync.dma_start(out=sn, in_=sin[:, jsl])
            pstate[("trig", j)] = (cs, sn)

        # ---- constants (critical-path DMAs first) ----------------------
        wq_sb = consts.tile([128, 1024], f16)
        wk_sb = consts.tile([128, 1024], f16)
        wv_sb = consts.tile([128, 1024], f16)
        for sb, dram in ((wq_sb, wq), (wk_sb, wk), (wv_sb, wv)):
            nc.sync.dma_start(
                out=sb.rearrange("p (t m) -> p t m", m=128),
                in_=dram.rearrange("(t p) m -> p t m", p=128),
            )
        wo_sb = consts.tile([128, 1024], f16)
        nc.sync.dma_start(out=wo_sb, in_=wo)
        tri_sb = consts.tile([128, 128], f16)
        nc.sync.dma_start(out=tri_sb, in_=tri)
        zero_sb = consts.tile([128, 128], f16)
        nc.vector.memset(zero_sb, 0.0)
        # preload the exp activation table while the weight DMAs run, so
        # the ~2.7us ACT_TABLE_LOAD is off the first chunk's critical path
        pre = smallp.tile([1, 64], f32, tag="pre")
        nc.scalar.activation(pre, zero_sb[0:1, 0:64],
                             AF.Exp, scale=1.0)
        id_sb = consts.tile([128, 128], f32r)
        nc.sync.dma_start(out=id_sb, in_=iden)
        id16_sb = consts.tile([128, 128], f16)
        nc.sync.dma_start(out=id16_sb, in_=iden16)

        qT_sb = persist.tile([128, S], f16)  # RoPE'd q, [dk(2 heads), s]
        kT_sb = persist.tile([128, S], f16)
        v_sb = persist.tile([128, 32 * VS], f16)
        vv = v_sb.rearrange("p (t c) -> p t c", c=VS)
        ones32 = ones.rearrange("p (t o) -> p t o", o=2)[:, 0:32, :]
        nc.sync.dma_start(out=vv[:, :, 64:65], in_=ones32[:, :, 0:1])
        nc.sync.dma_start(out=vv[:, :, 129:130], in_=ones32[:, :, 1:2])

        def rope(ps, dst, j):
            """dst = ps * cos + swap(ps) * sin  (chunk j)."""
            cs, sn = pstate[("trig", j)]
            sw = ropep.tile([128, CH], f32, tag="sw")
            nc.vector.stream_shuffle(sw, ps, SWAP_MASK)
            t1 = ropep.tile([128, CH], f32, tag="t1")
            nc.vector.tensor_tensor(t1, ps, cs, OP.mult)
            t2 = ropep.tile([128, CH], f32, tag="t2")
            nc.vector.tensor_tensor(t2, sw, sn, OP.mult)
            nc.vector.tensor_tensor(dst, t1, t2, OP.add)

        def proj_u1(j):
            """q projection + RoPE(q)."""
            jsl = slice(j * CH, (j + 1) * CH)
            if ("xt", j) not in pstate:
                fetch_chunk(j)
            xt = pstate[("xt", j)]
            psq = ps_misc.tile([128, CH], f32, tag="m")
            for t in range(8):
                nc.tensor.matmul(psq, wq_sb[:, t * 128:(t + 1) * 128],
                                 xt[:, t * CH:(t + 1) * CH],
                                 start=(t == 0), stop=(t == 7))
            rope(psq, qT_sb[:, jsl], j)

        def proj_u2(j):
            """k projection + RoPE(k)."""
            jsl = slice(j * CH, (j + 1) * CH)
            xt = pstate[("xt", j)]
            psk = ps_misc.tile([128, CH], f32, tag="m")
            for t in range(8):
                nc.tensor.matmul(psk, wk_sb[:, t * 128:(t + 1) * 128],
                                 xt[:, t * CH:(t + 1) * CH],
                                 start=(t == 0), stop=(t == 7))
            rope(psk, kT_sb[:, jsl], j)
            del pstate[("trig", j)]

        def proj_u3(j):
            """vT projection + copy to SBUF."""
            xt = pstate.pop(("xt", j))
            psvT = ps_misc.tile([128, CH], f32, tag="m")
            for t in range(8):
                nc.tensor.matmul(psvT, wv_sb[:, t * 128:(t + 1) * 128],
                                 xt[:, t * CH:(t + 1) * CH],
                                 start=(t == 0), stop=(t == 7))
            vt = ropep.tile([128, CH], f16, tag="vt")
            nc.vector.tensor_copy(vt, psvT)
            pstate[("vt", j)] = vt

        def proj_u4(j):
            """PE-transpose vT -> v tiles in v_sb (with ones columns)."""
            vt = pstate.pop(("vt", j))
            pst = ps_misc.tile([128, CH], f16, tag="m")
            for st in range(4):
                nc.tensor.transpose(pst[:, st * 128:(st + 1) * 128],
                                    vt[:, st * 128:(st + 1) * 128], id16_sb)
            for st in range(4):
                cb = (4 * j + st) * VS
                nc.vector.tensor_copy(v_sb[:, cb:cb + 64],
                                      pst[:, st * 128:st * 128 + 64])
                nc.vector.tensor_copy(v_sb[:, cb + 65:cb + 129],
                                      pst[:, st * 128 + 64:st * 128 + 128])

        def fin_norm(j):
            """Normalize chunk j's PV output: attnN = pv[:, :64] / den.
            Runs only after ALL of chunk j's PV accumulation stopped —
            reading a PSUM bank while the PE still accumulates other
            columns of the same bank corrupts data on hardware."""
            pvTA, pvTB = pstate.pop(("pvT", j))
            rcs = []
            for pvT in (pvTA, pvTB):
                dens = pvT.rearrange("p (s c) -> p s c", c=65)[:, :, 64:65]
                rc = smallp.tile([128, 4, 1], f32, tag="rc")
                nc.vector.reciprocal_approx_fast(out=rc, in_=dens)
                rcs.append(rc)
            for sub in range(4):
                an = attnp.tile([128, 128], f16, tag="an")
                for h, pvT in ((0, pvTA), (1, pvTB)):
                    nc.vector.tensor_scalar(
                        an[:, h * 64:h * 64 + 64],
                        pvT[:, sub * 65:sub * 65 + 64],
                        rcs[h][:, sub, :], None, OP.mult)
                pstate[("an", j, sub)] = an

        def fin_tr(j, st):
            """Transpose attnN for 128 output rows back to [d, q]."""
            an = pstate.pop(("an", j, st))
            trp = ps_misc.tile([128, CH], f16, tag="m")
            nc.tensor.transpose(trp[:, 0:128], an, id16_sb)
            outTs = outTp.tile([128, 128], f16, tag="oT")
            nc.vector.tensor_copy(outTs, trp[:, 0:128])
            pstate[("oT", j, st)] = outTs

        def fin_out(j, st):
            """Out-projection + store for 128 output rows of chunk j."""
            outTs = pstate.pop(("oT", j, st))
            stg = stagep.tile([128, 1024], f16, tag="stg")
            for oc in range(2):
                ops = ps_misc.tile([128, CH], f32, tag="m")
                nc.tensor.matmul(ops, outTs, wo_sb[:, oc * 512:(oc + 1) * 512],
                                 start=True, stop=True)
                nc.vector.tensor_copy(stg[:, oc * 512:(oc + 1) * 512], ops)
            r0 = j * CH + st * 128
            nc.sync.dma_start(out=out[r0:r0 + 128, :], in_=stg)

        def fin_st(j, st):
            fin_tr(j, st)
            fin_out(j, st)

        def attn_scores(j, t):
            """Scores for k-tile t against chunk j's queries (heads A||B via
            row tiling, packed side by side in one PSUM tile), then exp on
            ACT into one fp16 SBUF tile (single ACTIVATE when non-diag).
            Idempotent per (chunk-local) tile so transition pre-emission and
            the in-loop emission compose."""
            if ("e", t) in pstate:
                return
            jsl = slice(j * CH, (j + 1) * CH)
            off = max(0, 128 * (t - 4 * j))
            diag = t >= 4 * j
            ksl = slice(t * 128, (t + 1) * 128)
            sAB = ps_scores.tile([128, 2 * CH], f32, tag="s")
            nc.tensor.matmul(sAB[:, off:CH], kT_sb[0:64, ksl],
                             qT_sb[0:64, jsl][:, off:CH],
                             start=True, stop=True)
            nc.tensor.matmul(sAB[:, CH + off:2 * CH], kT_sb[64:128, ksl],
                             qT_sb[64:128, jsl][:, off:CH],
                             start=True, stop=True)
            eAB = expp.tile([128, 2 * CH], f16, tag="e")
            if not diag:
                nc.scalar.activation(eAB, sAB, AF.Exp, scale=0.125)
            else:
                nc.scalar.activation(eAB[:, off:CH], sAB[:, off:CH], AF.Exp,
                                     scale=0.125)
                nc.scalar.activation(eAB[:, CH + off:2 * CH],
                                     sAB[:, CH + off:2 * CH], AF.Exp,
                                     scale=0.125)
                for hb in (0, CH):
                    tm = slice(hb + off, hb + off + 128)
                    nc.vector.tensor_tensor(eAB[:, tm], eAB[:, tm], tri_sb,
                                            OP.mult)
            pstate[("e", t)] = eAB

        def attn_pvt(j, t, pvTA, pvTB):
            """PV-transposed accumulation for k-tile t into chunk j's
            per-query-subtile PSUM accumulators."""
            eAB = pstate.pop(("e", t))
            sub0 = max(0, t - 4 * j)
            for hb, pvT, vcol in ((0, pvTA, 0), (CH, pvTB, 65)):
                for sub in range(sub0, 4):
                    nc.tensor.matmul(
                        pvT[:, sub * 65:sub * 65 + 65],
                        eAB[:, hb + sub * 128:hb + sub * 128 + 128],
                        v_sb[:, t * VS + vcol:t * VS + vcol + 65],
                        start=False, stop=(t == 4 * j + sub),
                        skip_group_check=True)

        PROJ_UNITS = (proj_u1, proj_u2, proj_u3, proj_u4)

        for rep in range(repeats):
            for u in PROJ_UNITS:
                u(0)
            for j in range(NCH):
                if j + 1 < NCH:
                    # prefetch the next chunk's x/trig now: the DMAs get a
                    # full chunk of lead time instead of being issued right
                    # before proj_u1(j+1)'s matmuls consume them.
                    fetch_chunk(j + 1)
                pvTA = ps_pv.tile([128, 260], f32, tag="pvA")
                pvTB = ps_pv.tile([128, 260], f32, tag="pvB")
                pstate[("pvT", j)] = (pvTA, pvTB)
                tiles = list(range(4 * j + 4))
                units = []
                if j + 1 < NCH:
                    # q/k projections first so the next chunk's scores can
                    # issue immediately at the transition
                    units.append(lambda jj=j + 1: proj_u1(jj))
                    units.append(lambda jj=j + 1: proj_u2(jj))
                if j > 0:
                    for st in range(4):
                        units.append(lambda jj=j - 1, ss=st: fin_st(jj, ss))
                if j + 1 < NCH:
                    units.append(lambda jj=j + 1: proj_u3(jj))
                    units.append(lambda jj=j + 1: proj_u4(jj))
                ng = len(tiles)
                done = 0
                attn_scores(j, 0)
                if ng > 1:
                    attn_scores(j, 1)
                for ti, t in enumerate(tiles):
                    # scores run two tiles ahead of this tile's PV so ACT
                    # always has the next exp queued and never starves
                    # while the PE waits on PV dependencies.
                    if t + 2 < ng:
                        attn_scores(j, t + 2)
                    if ti == 0:
                        # zero the PV psum accumulators with in-stream PE
                        # matmuls (zero stationary); all PV matmuls then
                        # accumulate with start=False (a per-subtile start
                        # would zero the whole 2KB bank and clobber sibling
                        # subtile accumulators). Emitted AFTER the scores
                        # lookahead: these wait on the previous chunk's
                        # fin_norm reads, and ahead of the scores they
                        # would head-block the PE queue and starve ACT at
                        # every chunk boundary.
                        for pvT in (pvTA, pvTB):
                            nc.tensor.matmul(pvT, zero_sb, wq_sb[:, 0:260],
                                             start=True, stop=False,
                                             skip_group_check=True)
                    attn_pvt(j, t, pvTA, pvTB)
                    want = (ti + 1) * len(units) // ng
                    while done < want:
                        units[done]()
                        done += 1
                # normalize as soon as this chunk's accumulation is complete:
                # frees the PV psum banks so the next chunk's zeroing matmuls
                # don't stall the PE queue on a late cross-engine WAR.
                fin_norm(j)
                # pre-emit the next chunk's first two score tiles (q/k were
                # projected mid-chunk) so ACT rolls into the next chunk's
                # exp stream with no transition gap.
                if j + 1 < NCH:
                    attn_scores(j + 1, 0)
                    attn_scores(j + 1, 1)

            for st in range(4):
                fin_tr(NCH - 1, st)
            for st in range(4):
                fin_out(NCH - 1, st)


def _build(repeats=1):
    import concourse.mybir as mybir
    import concourse.tile as tile
    from concourse import bacc

    f32 = mybir.dt.float32
    f16 = mybir.dt.float16
    nc = bacc.Bacc("TRN2", target_bir_lowering=False, debug=False,
                   num_devices=NCORES)
    aps = {}
    for name, shape in (
        ("xT", [D, S]), ("wq", [D, 128]), ("wk", [D, 128]), ("wv", [D, 128]),
        ("wo", [128, D]), ("ones", [128, 64]), ("tri", [128, 128]),
    ):
        aps[name] = nc.dram_tensor(name, shape, f16, kind="ExternalInput").ap()
    for name, shape in (
        ("cos", [128, S]), ("sin", [128, S]),
    ):
        aps[name] = nc.dram_tensor(name, shape, f32, kind="ExternalInput").ap()
    aps["iden"] = nc.dram_tensor(
        "iden", [128, 128], mybir.dt.float32r, kind="ExternalInput").ap()
    aps["iden16"] = nc.dram_tensor(
        "iden16", [128, 128], f16, kind="ExternalInput").ap()
    out_ap = nc.dram_tensor("out", [S, D], f16, kind="ExternalOutput").ap()

    with tile.TileContext(nc) as tc:
        _emit(tc, out_ap, aps["xT"], aps["wq"], aps["wk"], aps["wv"],
              aps["wo"], aps["cos"], aps["sin"], aps["tri"], aps["ones"],
              aps["iden"], aps["iden16"], repeats=repeats)
    nc.compile()
    return nc


def kernel(x, Wq, Wk, Wv, Wo):
    from concourse.bass_utils import run_bass_kernel_spmd

    if "nc" not in _CACHE:
        _CACHE["nc"] = _build()
    nc = _CACHE["nc"]

    in_maps = _host_prep(x, Wq, Wk, Wv, Wo)
    res = run_bass_kernel_spmd(nc, in_maps, core_ids=list(range(NCORES)))
    acc = np.zeros((S, D), dtype=np.float64)
    for r in res.results:
        acc += r["out"].astype(np.float64)
    return acc.astype(np.float32).reshape(1, S, D)

